# revision 1
# baseline (speedup 1.0000x reference)
"""NetVLAD Trainium2 Bass kernel, SPMD over 8 NeuronCores.

Contract: kernel(x, Wc, C) takes the FULL inputs
  x  [16, 56, 56, 512] f32, Wc [512, 32] f32, C [512, 32] f32
and returns the FULL output [16, 16384] f32 (matches reference()).

Sharding: data-parallel over batch - 2 samples per core; Wc/C replicated.

Design (3136 pixels/sample, D=512, K=32; 49 tiles of 128 pixels):
  - x, Wc, C are cast to bf16 on the HOST, halving the HBM stream
    (6.4 MB/core); all matmul paths run bf16 with f32 PSUM accumulation
    (end-to-end rel err ~2.3e-3 vs the f32 reference, budget 2e-2)
  - all of x is DMA'd up front into SBUF (15 chunked DMAs, first ones
    small) -> zero DMA backpressure; constants stream in parallel on
    the gpsimd SWDGE queue; the Exp+Ln activation table is pinned once
  - both samples' VLAD accumulators live in ONE PSUM bank as [d, 4, k]
    regions (plus an asum bank), opened ONCE by a start=True matmul of
    DMA'd zeros covering every region; all real matmuls accumulate
    with start=False (the only start pattern whose semantics agree
    between HW per-element has_written and CoreSim's 2KB zero-region
    model). NOTE: dram params declared bf16 MUST receive bf16 numpy
    arrays from the host - f32 arrays are silently reinterpreted on
    the PJRT path and produce NaN.
  - software-pipelined loop, per iteration t:
      stage C (t-4): PE mm2 acc[d,k] += x_chunk.T @ a (4 matmuls,
                     x-chunk stationary / a moving, output born in the
                     final [d, k] layout) and a_sum += a.T @ ones
      stage A (t):   PE transposes x tile -> xT in PSUM (bf16 PSUM)
      stage A' :     single DVE copy drains xT PSUM->SBUF (bf16 2x
                     packed mode, ~440 ns)
      stage B (t-2): PE mm1 s = xT.T @ Wc (4 accumulating bf16
                     matmuls); ACT Exp with fused row-sum accumulator;
                     GPSIMD normalize_recip does the softmax divide
                     (SBUF-only op on the otherwise idle Pool engine)
  - per-sample epilogue (no PSUM drain, no back-transpose):
      pre:  DVE builds diag(asum) = id32 * asum
      post: 4 PE matmuls fold the C*asum term straight into the
            accumulator (acc += ct_chunk.T @ diag, carrying the stop);
            ACT Square + DVE reduce for ssq; rmult = exp(-0.5*ln(D*ssq))
            = 1/sqrt(D*ssq) on ACT (the global L2 norm of the
            intra-normalized matrix is exactly sqrt(D), folded into the
            ln scale); DVE scales straight out of PSUM; output DMA'd in
            two halves on two queues
CoreSim: 30.0 us/core (baseline 79.5); HW-validated rel err 2.3e-3.
"""
import sys

if '/opt/trn_rl_repo' not in sys.path:
    sys.path.insert(0, '/opt/trn_rl_repo')

from contextlib import ExitStack

import numpy as np

N_PIX = 3136
N_SAMP = 2
N_ROWS = N_PIX * N_SAMP
P = 128
NT = N_ROWS // P      # 49
D = 512
K = 32
DC = D // P           # 4
BOUND_T = N_PIX // P  # 24
BOUND_R = N_PIX - BOUND_T * P  # 64
N_CORES = 8

_cache = {}


def _build(bf16_mm1=True):
    import concourse.bacc as bacc
    import concourse.mybir as mybir
    import concourse.tile as tile
    from concourse.bass import ts

    F32 = mybir.dt.float32
    F32R = mybir.dt.float32r
    BF16 = mybir.dt.bfloat16
    MDT = BF16 if bf16_mm1 else F32R

    nc = bacc.Bacc("TRN2", target_bir_lowering=False, debug=False)

    x = nc.declare_dram_parameter("x", [N_ROWS, D], BF16, isOutput=False)
    wc = nc.declare_dram_parameter("wc", [D, K], BF16, isOutput=False)
    ct = nc.declare_dram_parameter("ct", [K, D], BF16, isOutput=False)
    ident = nc.declare_dram_parameter("ident", [P, P], BF16, isOutput=False)
    id32 = nc.declare_dram_parameter("id32", [K, K], BF16, isOutput=False)
    zeros = nc.declare_dram_parameter("zeros", [P, 2 * DC * K], BF16,
                                      isOutput=False)
    ones2 = nc.declare_dram_parameter("ones2", [P, 2], BF16, isOutput=False)
    out = nc.declare_dram_parameter("out", [N_SAMP, DC, P, K], F32,
                                    isOutput=True)
    x, wc, ct, ident, out, id32, ones2, zeros = (
        x.ap(), wc.ap(), ct.ap(), ident.ap(), out.ap(), id32.ap(),
        ones2.ap(), zeros.ap())

    with tile.TileContext(nc) as tc, ExitStack() as ctx:
        consts = ctx.enter_context(tc.tile_pool(name="consts", bufs=1))
        xbig = ctx.enter_context(tc.tile_pool(name="xbig", bufs=1))
        xtpool = ctx.enter_context(tc.tile_pool(name="xtpool", bufs=8))
        small = ctx.enter_context(tc.tile_pool(name="small", bufs=10))
        epil = ctx.enter_context(tc.tile_pool(name="epil", bufs=2))
        ps_big = ctx.enter_context(tc.tile_pool(name="ps_big", bufs=3,
                                                space="PSUM"))
        ps_sm = ctx.enter_context(tc.tile_pool(name="ps_sm", bufs=3,
                                               space="PSUM"))
        ps_acc = ctx.enter_context(tc.tile_pool(name="ps_acc", bufs=1,
                                                space="PSUM"))

        # constants on the gpsimd SWDGE queue (parallel with the x
        # stream on SP); id_sb first since transpose(0) needs it
        id_sb = consts.tile([P, P], BF16)
        nc.gpsimd.dma_start(out=id_sb, in_=ident)
        wc_sb = consts.tile([P, DC, K], BF16)
        nc.gpsimd.dma_start(out=wc_sb, in_=wc.rearrange("(c p) k -> p c k",
                                                        p=P))
        ones_sb = consts.tile([P, 2], BF16)
        nc.gpsimd.dma_start(out=ones_sb, in_=ones2)
        ct_sb = consts.tile([K, D], BF16)
        nc.gpsimd.dma_start(out=ct_sb, in_=ct)
        id32_sb = consts.tile([K, K], BF16)
        nc.gpsimd.dma_start(out=id32_sb, in_=id32)
        zeros_sb = consts.tile([P, 2 * DC * K], BF16)
        nc.gpsimd.dma_start(out=zeros_sb, in_=zeros)

        # pin the one act table covering Exp+Ln up front (overlaps the
        # initial DMA) so the fixpoint pass doesn't thrash between the
        # exp-only and ln-only sets at the epilogues; purely a perf hint,
        # so degrade gracefully if the table metadata isn't findable
        try:
            from concourse.hw_specs import get_activation_tables
            sets = get_activation_tables(nc.m.arch)
            _EXP = mybir.ActivationFunctionType.Exp
            _LN = mybir.ActivationFunctionType.Ln
            set_id = next(i for i, name in enumerate(sets)
                          if _EXP in sets[name] and _LN in sets[name])
            nc.scalar.add_instruction(
                mybir.InstLoadActFuncSet(
                    name=nc.get_next_instruction_name(), ins=[], outs=[],
                    act_func_set_id=set_id))
        except Exception:
            pass

        # whole x resident in SBUF; first chunks small so compute starts
        # ~800ns in, then 4-tile DMAs stream at roofline with no
        # backpressure
        chunk_sizes = [1, 1, 2] + [4] * ((NT - 4) // 4)
        rem = NT - sum(chunk_sizes)
        if rem:
            chunk_sizes.append(rem)
        xc = []
        tile_of = []
        r0 = 0
        for c, n in enumerate(chunk_sizes):
            t_ = xbig.tile([P, n, D], BF16, name=f"xc{c}")
            if c == 0:
                # two half-d DMAs: transposes 0-1 start as soon as the
                # first 256 columns land
                for h in range(2):
                    hd = slice(h * D // 2, (h + 1) * D // 2)
                    nc.sync.dma_start(out=t_[:, 0, hd],
                                      in_=x[r0 * P:(r0 + n) * P, hd])
            else:
                nc.sync.dma_start(
                    out=t_,
                    in_=x[r0 * P:(r0 + n) * P, :].rearrange(
                        "(j p) d -> p j d", p=P))
            xc.append(t_)
            for j in range(n):
                tile_of.append((c, j))
            r0 += n

        def xview(t):
            c, j = tile_of[t]
            return xc[c][:, j, :]

        # one PSUM bank holds both samples' [d, 4, k] accumulators in
        # disjoint column ranges; a second holds the two asum pairs. The
        # banks are opened ONCE by a start=True matmul of DMA'd zeros
        # covering every region (per-element has_written set on HW, all
        # zero-region marks consumed in CoreSim); everything after
        # accumulates with start=False.
        ps_all = ps_acc.tile([P, 2 * DC * K], F32, name="ps_all", tag="acc")
        asums = ps_acc.tile([K, 4], F32, name="asums", tag="asum")
        nc.tensor.matmul(ps_all, id_sb, zeros_sb,
                         start=True, stop=False, skip_group_check=True)
        nc.tensor.matmul(asums, id_sb[:, 0:K], zeros_sb[:, 0:4],
                         start=True, stop=False, skip_group_check=True)

        def acc_jk(s, j):
            o = (s * DC + j) * K
            return ps_all[:, o:o + K]

        def acc_hj(s, h):
            o = (s * DC + 2 * h) * K
            return ps_all[:, o:o + 2 * K]

        asum_ps = [asums[:, 2 * s:2 * s + 2] for s in range(N_SAMP)]

        diags = {}

        def epilogue_pre(s):
            asum_sb = epil.tile([K, 1], F32, name=f"asum{s}", tag="asum")
            nc.vector.tensor_copy(asum_sb, asum_ps[s][:, 0:1])
            # diag(asum) = id32 * asum (per-partition scalar), bf16 for PE
            diag = epil.tile([K, K], BF16, name=f"diag{s}", tag="diag")
            nc.vector.tensor_scalar_mul(diag, id32_sb, asum_sb)
            diags[s] = diag

        def epilogue_post(s):
            # fold the C*asum term into the PSUM accumulation:
            # acc[s] chunk j += ct_chunk.T @ diag(asum); these carry the
            # stop of the accumulation groups
            diag = diags.pop(s)
            for j in range(DC):
                nc.tensor.matmul(acc_jk(s, j), ct_sb[:, ts(j, P)], diag,
                                 start=False, stop=True,
                                 skip_group_check=True)
            v_sb = epil.tile([P, DC, K], F32, name=f"v{s}", tag="v")
            vsq = epil.tile([P, DC, K], F32, name=f"vsq{s}", tag="vsq")
            ssq = epil.tile([P, DC], F32, name=f"ssq{s}", tag="ssq")
            lssq = epil.tile([P, DC], F32, name=f"ls{s}", tag="ls")
            rmult = epil.tile([P, DC], F32, name=f"rm{s}", tag="rm")
            # rmult = exp(-0.5*ln(D*ssq)) = 1/sqrt(D*ssq); ln+exp share
            # one act func set (no Sqrt set switch)
            for h in range(2):
                hj = slice(2 * h, 2 * h + 2)
                nc.scalar.activation(vsq[:, hj, :], acc_hj(s, h),
                                     mybir.ActivationFunctionType.Square)
                nc.vector.reduce_sum(ssq[:, hj], vsq[:, hj, :],
                                     axis=mybir.AxisListType.X)
            nc.scalar.activation(lssq, ssq,
                                 mybir.ActivationFunctionType.Ln,
                                 scale=float(D))
            nc.scalar.activation(rmult, lssq,
                                 mybir.ActivationFunctionType.Exp,
                                 scale=-0.5)
            for h in range(2):
                hj = slice(2 * h, 2 * h + 2)
                for j in (2 * h, 2 * h + 1):
                    nc.vector.tensor_scalar_mul(v_sb[:, j, :],
                                                acc_jk(s, j),
                                                rmult[:, j:j + 1])
                qeng = nc.sync if h == 1 else nc.gpsimd
                qeng.dma_start(
                    out=out[s][hj].rearrange("c p k -> p c k"),
                    in_=v_sb[:, hj, :])

        xT_sbs = {}
        a_sbs = {}

        def stage_a(t):
            xv = xview(t)
            xT_ps = ps_big.tile([P, DC, P], BF16, name="xT_ps")
            xT_sb = xtpool.tile([P, DC, P], MDT, name="xT_sb")
            # PSUM->SBUF drain: one DVE instr (gpsimd has no PSUM port;
            # per-instr PSUM latency makes one copy cheaper than two);
            # mm1 consumes it two iterations later so the chain never
            # stalls PE
            for j in range(DC):
                nc.tensor.transpose(xT_ps[:, j, :], xv[:, ts(j, P)], id_sb)
            nc.vector.tensor_copy(xT_sb, xT_ps)
            xT_sbs[t] = xT_sb

        def stage_b(t):
            xT_sb = xT_sbs.pop(t)
            s_ps = ps_sm.tile([P, K], F32, name="s_ps", tag="sps")
            for j in range(DC):
                nc.tensor.matmul(s_ps, xT_sb[:, j, :], wc_sb[:, j, :],
                                 start=(j == 0), stop=(j == DC - 1))
            exp_sb = small.tile([P, K], F32, name="exp_sb")
            sumx = small.tile([P, 1], F32, name="sumx")
            nc.scalar.activation(exp_sb, s_ps,
                                 mybir.ActivationFunctionType.Exp,
                                 accum_out=sumx)
            a_sb = small.tile([P, K], BF16, name="a_sb")
            # softmax divide on the idle GPSIMD/Pool engine (SBUF-only op)
            nc.gpsimd.normalize_recip(a_sb, exp_sb, sumx)
            a_sbs[t] = a_sb

        def stage_c(t):
            a_sb = a_sbs.pop(t)
            xv = xview(t)
            if t < BOUND_T:
                parts = [(0, 0, P)]
            elif t == BOUND_T:
                parts = [(0, 0, BOUND_R), (1, BOUND_R, P)]
            else:
                parts = [(1, 0, P)]
            for s, r0, r1 in parts:
                last_tile = (t == BOUND_T and s == 0) or \
                            (t == NT - 1 and s == 1)
                for j in range(DC):
                    # acc[d, k] += x_chunk.T @ a -- output born in [d, k]
                    # layout, no epilogue back-transpose needed; the
                    # C*asum matmuls carry the stop
                    nc.tensor.matmul(acc_jk(s, j),
                                     xv[r0:r1, ts(j, P)], a_sb[r0:r1, :],
                                     start=False, stop=False,
                                     skip_group_check=True)
                nc.tensor.matmul(asum_ps[s][:, :], a_sb[r0:r1, :],
                                 ones_sb[r0:r1, :],
                                 start=False, stop=last_tile,
                                 skip_group_check=True)
                if last_tile:
                    epilogue_pre(s)

        for t in range(NT + 7):
            if 0 <= t - 4 < NT:
                stage_c(t - 4)
            if t < NT:
                stage_a(t)
            if 0 <= t - 2 < NT:
                stage_b(t - 2)
            if t - 6 == BOUND_T:
                epilogue_post(0)
            if t - 6 == NT - 1:
                epilogue_post(1)

    nc.finalize()
    return nc


def _get_nc():
    if "nc" not in _cache:
        _cache["nc"] = _build()
    return _cache["nc"]


def _make_maps(x, Wc, C):
    import ml_dtypes
    bf16 = ml_dtypes.bfloat16
    x = np.asarray(x, dtype=np.float32).astype(bf16)
    Wc = np.ascontiguousarray(np.asarray(Wc, dtype=np.float32).astype(bf16))
    ct = np.ascontiguousarray(np.asarray(C, dtype=np.float32).T.astype(bf16))
    ident = np.eye(P, dtype=np.float32).astype(bf16)
    id32 = np.eye(K, dtype=np.float32).astype(bf16)
    ones2 = np.ones((P, 2), dtype=np.float32).astype(bf16)
    zeros = np.zeros((P, 2 * DC * K), dtype=np.float32).astype(bf16)

    B = x.shape[0]
    per = B // N_CORES
    maps = []
    for i in range(N_CORES):
        xs = np.ascontiguousarray(
            x[i * per:(i + 1) * per].reshape(N_ROWS, D))
        maps.append({"x": xs, "wc": Wc, "ct": ct, "ident": ident,
                     "id32": id32, "ones2": ones2, "zeros": zeros})
    return maps


def kernel(x, Wc, C):
    from concourse.bass_utils import run_bass_kernel_spmd

    nc = _get_nc()
    maps = _make_maps(x, Wc, C)
    res = run_bass_kernel_spmd(nc, maps, list(range(N_CORES)))
    outs = [r["out"].reshape(N_SAMP, D * K) for r in res.results]
    return np.concatenate(outs, axis=0)

